# revision 32
# baseline (speedup 1.0000x reference)
"""Bahdanau-attention kernel for Trainium2, data-parallel over 8 NeuronCores.

Per core (B_local=8, T=1024, H=1024), per batch b:
  encT[o,t] = sum_h W_enc[o,h] * x[t,h]   (PE: K_PAIR h-chunks as fp8 e4m3
      DoubleRow pairs -- 2 K-tiles of 128 per call at 2 cols/cycle -- the
      rest as exact bf16 calls. All W scaled by 256 -> tanh scale 2^-8.)
  energyT   = tanh(encT*2^-8 + (W_dec h + b_dec + b_enc)[o])  (ScalarE)
  scores[t] = sum_o w_score[o] * energyT[o,t]   (PE, bf16)
  weights   = softmax(scores)   (exp on ScalarE straight from PSUM,
      weight columnization via K=1 matmuls against a ones row)
  context   = sum_t weights[t] * x[t,:]         (PE, bf16 natural-layout x)
b_score dropped: softmax shift-invariance cancels it in both outputs.
All transposes are done host-side (xT fp8 pairs, W_encT fp8, W_decT/hsT
bf16, x natural bf16) -- no PE transposes in steady state.
"""

import sys

if "/opt/trn_rl_repo" not in sys.path:
    sys.path.insert(0, "/opt/trn_rl_repo")

import numpy as np

B, T, H = 64, 1024, 1024
NCORES = 8
BL = B // NCORES
P = 128
NT = T // P  # t chunks
NH = H // P  # h chunks
NO = H // P  # o blocks
F = 512      # PSUM bank free size (f32)
NS = T // F
K_PAIR = 6             # h-chunks contracted as fp8 DoubleRow pairs
NPAIR = K_PAIR // 2    # fp8 pair calls (2 chunks each)
NBF = NH - K_PAIR      # remaining h-chunks in exact bf16

_CACHE = {}
LAST_RESULT = None


def build(bl=BL):
    import concourse.tile as tile
    from concourse import bacc, mybir
    from concourse.masks import make_identity

    f32 = mybir.dt.float32
    f8 = mybir.dt.float8e4
    bf = mybir.dt.bfloat16
    AF = mybir.ActivationFunctionType
    DR = mybir.MatmulPerfMode.DoubleRow

    nc = bacc.Bacc("TRN2", target_bir_lowering=False, debug=False, num_devices=NCORES)
    xp_d = nc.declare_dram_parameter("x_pairs", [bl, P, NPAIR, 2, T], f8, isOutput=False)
    xb_d = nc.declare_dram_parameter("x_bf", [bl, P, NBF, T], bf, isOutput=False)
    xn_d = nc.declare_dram_parameter("x_nat", [bl, P, NT, H], bf, isOutput=False)
    wp_d = nc.declare_dram_parameter("w_pairs", [P, NO, NPAIR, 2, P], f8, isOutput=False)
    wb_d = nc.declare_dram_parameter("w_bf", [P, NBF, H], bf, isOutput=False)
    wdT_d = nc.declare_dram_parameter("wdT", [P, NH, H], bf, isOutput=False)
    hsT_d = nc.declare_dram_parameter("hsT", [P, NH * bl], bf, isOutput=False)
    ws_d = nc.declare_dram_parameter("ws_col", [P, NH], bf, isOutput=False)
    be_d = nc.declare_dram_parameter("be_col", [P, NH], f32, isOutput=False)
    bd_d = nc.declare_dram_parameter("bd_col", [P, NH], f32, isOutput=False)
    ctx_d = nc.declare_dram_parameter("out_ctx", [bl, H], f32, isOutput=True)
    wout_d = nc.declare_dram_parameter("out_w", [bl, T], f32, isOutput=True)

    with tile.TileContext(nc) as tc:
        with (
            tc.tile_pool(name="const", bufs=1) as const,
            tc.tile_pool(name="xp", bufs=2) as xp_pool,
            tc.tile_pool(name="xn", bufs=3) as xn_pool,
            tc.tile_pool(name="eT", bufs=6) as eT_pool,
            tc.tile_pool(name="rows", bufs=2) as rows,
            tc.tile_pool(name="small", bufs=2) as small,
            tc.tile_pool(name="mmps", bufs=4, space="PSUM") as mm_ps,
            tc.tile_pool(name="scps", bufs=2, space="PSUM") as sc_ps,
            tc.tile_pool(name="ctxps", bufs=1, space="PSUM") as ctx_ps,
            tc.tile_pool(name="decps", bufs=1, space="PSUM") as dec_ps,
        ):
            # ---- constants built first (gpsimd/vector heads, before DMA issues) ----
            ident_f = const.tile([P, P], f32, tag="identf")
            make_identity(nc, ident_f[:])
            ones_f = const.tile([1, NT], f32, tag="onesf")
            nc.vector.memset(ones_f[:], 1.0)
            ones8 = const.tile([1, NT], bf, tag="ones8")
            nc.vector.tensor_copy(ones8[:], ones_f[:])

            # ---- weights/batch-0 DMA ----
            # DMA bw (~220GB/s) is striped across hw engines and shared
            # proportionally among active queues: schedule two symmetric
            # queues in global priority order (enc-pair operands, wdT for the
            # dec bias, bf16-chunk operands, then batch-0 x_nat).
            hsT = const.tile([P, NH * bl], bf, tag="hsT")
            nc.sync.dma_start(hsT[:], hsT_d[:, :])
            xp0 = xp_pool.tile([P, NPAIR, 2, T], f8, tag="xp")
            nc.sync.dma_start(xp0[:], xp_d[0, :, :, :, :])
            wTt = const.tile([P, NO, NPAIR, 2, P], f8, tag="wenc")
            nc.scalar.dma_start(wTt[:], wp_d[:, :, :, :, :])
            ws_sb = const.tile([P, NH], bf, tag="ws")
            nc.gpsimd.dma_start(ws_sb[:], ws_d[:, :])
            be_sb = const.tile([P, NH], f32, tag="be")
            nc.gpsimd.dma_start(be_sb[:], be_d[:, :])
            bd_sb = const.tile([P, NH], f32, tag="bd")
            nc.gpsimd.dma_start(bd_sb[:], bd_d[:, :])
            wdTt = const.tile([P, NH, H], bf, tag="wdT")
            nc.gpsimd.dma_start(wdTt[:, 4:8, :], wdT_d[:, 4:8, :])
            nc.sync.dma_start(wdTt[:, 2:4, :], wdT_d[:, 2:4, :])
            nc.scalar.dma_start(wdTt[:, 0:2, :], wdT_d[:, 0:2, :])
            xb0 = xp_pool.tile([P, NBF, T], bf, tag="xb")
            nc.sync.dma_start(xb0[:], xb_d[0, :, :, :])
            wBt = const.tile([P, NBF, H], bf, tag="wbf")
            nc.scalar.dma_start(wBt[:], wb_d[:, :, :])
            bsum = const.tile([P, NH], f32, tag="bsum")
            nc.vector.tensor_add(bsum[:], be_sb[:], bd_sb[:])

            # ---- PE warmup: keep the array busy so HAM ramps during DMA wait
            warm_ps = mm_ps.tile([P, P], f32, tag="mmps", name="warm")
            for _ in range(16):
                nc.tensor.transpose(warm_ps[:], ident_f[:], ident_f[:])

            # ---- per-batch x DMA ----
            def dma_x(it):
                xp_t = xp_pool.tile([P, NPAIR, 2, T], f8, tag="xp")
                nc.sync.dma_start(xp_t[:], xp_d[it, :, :, :, :])
                xb_t = xp_pool.tile([P, NBF, T], bf, tag="xb")
                nc.sync.dma_start(xb_t[:], xb_d[it, :, :, :])
                xn_t = xn_pool.tile([P, NT, H], bf, tag="xn")
                nc.gpsimd.dma_start(xn_t[:, 0:4, :], xn_d[it, :, 0:4, :])
                nc.gpsimd.dma_start(xn_t[:, 4:8, :], xn_d[it, :, 4:8, :])
                return (xp_t, xb_t), xn_t

            xp_cur = (xp0, xb0)
            xn0 = xn_pool.tile([P, NT, H], bf, tag="xn")
            nc.gpsimd.dma_start(xn0[:, 0:4, :], xn_d[0, :, 0:4, :])
            nc.scalar.dma_start(xn0[:, 4:8, :], xn_d[0, :, 4:8, :])
            xn_cur = xn0

            bias_all = const.tile([P, NO * bl], f32, tag="bias_all")

            # ---- helpers ----
            def enc_mm(ob, x_cur):
                """enc matmuls for one o-block; returns PSUM pair."""
                xp_t, xb_t = x_cur
                psE = [mm_ps.tile([P, F], f32, tag="mmps", name=f"psE{ob}_{h}")
                       for h in range(NS)]
                for c in range(NPAIR):
                    for half in range(NS):
                        nc.tensor.matmul(
                            psE[half][:],
                            wTt[:, ob, c, :, :],
                            xp_t[:, c, :, half * F : (half + 1) * F],
                            start=(c == 0),
                            stop=False,
                            perf_mode=DR,
                        )
                for j in range(NBF):
                    for half in range(NS):
                        nc.tensor.matmul(
                            psE[half][:],
                            wBt[:, j, ob * P : (ob + 1) * P],
                            xb_t[:, j, half * F : (half + 1) * F],
                            start=False,
                            stop=(j == NBF - 1),
                        )
                return psE

            def enc_act(it, ob, psE):
                es = []
                for half in range(NS):
                    e = eT_pool.tile([P, F], bf, tag="eT")
                    nc.scalar.activation(
                        e[:], psE[half][:], AF.Tanh,
                        bias=bias_all[:, ob * bl + it : ob * bl + it + 1],
                        scale=1.0 / 256.0,
                    )
                    es.append(e)
                return es

            def enc_ob(it, ob, x_cur):
                return enc_act(it, ob, enc_mm(ob, x_cur))

            def dec_bias():
                """dec[b,o] = sum_h hs[b,h] Wd[o,h]; transpose into bias_all."""
                dec_sb = const.tile([bl, H], f32, tag="decsb")
                psd = [sc_ps.tile([P, F], f32, tag="scps", name=f"psd{h}")
                       for h in range(NS)]
                for c in range(NH):
                    for half in range(NS):
                        nc.tensor.matmul(
                            psd[half][0:bl, :],
                            hsT[:, c * bl : (c + 1) * bl],
                            wdTt[:, c, half * F : (half + 1) * F],
                            start=(c == 0),
                            stop=(c == NH - 1),
                        )
                for half in range(NS):
                    nc.vector.tensor_copy(
                        dec_sb[:, half * F : (half + 1) * F], psd[half][0:bl, :]
                    )
                psT = dec_ps.tile([P, NO * bl], f32, tag="decps")
                for ob in range(NO):
                    nc.tensor.transpose(
                        psT[:, ob * bl : (ob + 1) * bl],
                        dec_sb[:, ob * P : (ob + 1) * P], ident_f[0:bl, 0:bl]
                    )
                for ob in range(NO):
                    nc.vector.tensor_scalar_add(
                        bias_all[:, ob * bl : (ob + 1) * bl],
                        psT[:, ob * bl : (ob + 1) * bl],
                        bsum[:, ob : ob + 1],
                    )

            def score_ob(ob, ets, pss):
                for half in range(NS):
                    nc.tensor.matmul(
                        pss[half][:],
                        ws_sb[:, ob : ob + 1],
                        ets[ob][half][:],
                        start=(ob == 0),
                        stop=(ob == NO - 1),
                    )

            # ---- main pipeline ----
            prev = None
            for it in range(bl + 1):
                if 0 < it < bl:
                    xp_cur, xn_cur = dma_x(it)

                if it == 0:
                    # batch-0 enc matmuls for ob0/ob1 run while wdT still
                    # streams in; dec lands in between, then tanh unblocks.
                    psE0 = enc_mm(0, xp_cur)
                    psE1 = enc_mm(1, xp_cur)
                    dec_bias()
                    ets = [enc_act(0, 0, psE0), enc_act(0, 1, psE1)]
                elif it < bl:
                    # o-block 0 first: gives PE work while prev softmax drains
                    ets = [enc_ob(it, 0, xp_cur)]

                if it >= 1:
                    st = prev
                    # columnize exp-weights: uT[:, c] = u[c*128:(c+1)*128]
                    psw = mm_ps.tile([P, NT * NT], f32, tag="mmps")
                    for c in range(NT):
                        nc.tensor.matmul(
                            psw[:, c * NT : (c + 1) * NT],
                            st["u_rowr"][0:1, c * P : (c + 1) * P],
                            ones8[:],
                            start=(c == 0),
                            stop=(c == NT - 1),
                        )
                    uT = small.tile([P, NT], bf, tag="uT")
                    nc.vector.tensor_copy(uT[:], psw[:, 0 : NT * NT : NT])
                    ctx_row = rows.tile([1, H], f32, tag="ctxrow")
                    for half in range(NS):
                        pc = ctx_ps.tile([1, F], f32, tag="ctxps")
                        for c in range(NT):
                            nc.tensor.matmul(
                                pc[:],
                                uT[:, c : c + 1],
                                st["xn"][:, c, half * F : (half + 1) * F],
                                start=(c == 0),
                                stop=(c == NT - 1),
                            )
                        nc.vector.tensor_scalar_mul(
                            ctx_row[0:1, half * F : (half + 1) * F],
                            pc[0:1, :],
                            st["rz"][0:1, 0:1],
                        )
                    nc.sync.dma_start(ctx_d[it - 1 : it, :], ctx_row[:])
                    w_row = rows.tile([1, T], f32, tag="wrow")
                    nc.vector.tensor_scalar_mul(
                        w_row[:], st["u_row"][:], st["rz"][0:1, 0:1]
                    )
                    nc.sync.dma_start(wout_d[it - 1 : it, :], w_row[:])

                if it < bl:
                    pss = [sc_ps.tile([1, F], f32, tag="scps", name=f"pss{h}")
                           for h in range(NS)]
                    scored = 0
                    for ob in range(len(ets), NO):
                        ets.append(enc_ob(it, ob, xp_cur))
                        score_ob(scored, ets, pss)
                        scored += 1
                    while scored < NO:
                        score_ob(scored, ets, pss)
                        scored += 1
                    # softmax pieces: exp straight from PSUM
                    u_rowr = rows.tile([1, T], bf, tag="urowr")
                    u_row = rows.tile([1, T], f32, tag="urow")
                    ssum = small.tile([1, NS], f32, tag="ssum")
                    for half in range(NS):
                        nc.scalar.activation(
                            u_rowr[0:1, half * F : (half + 1) * F],
                            pss[half][0:1, :], AF.Exp, bias=0.0, scale=1.0,
                        )
                    for half in range(NS):
                        nc.scalar.activation(
                            u_row[0:1, half * F : (half + 1) * F],
                            pss[half][0:1, :], AF.Exp, bias=0.0, scale=1.0,
                            accum_out=ssum[0:1, half : half + 1],
                        )
                    stot = small.tile([1, 1], f32, tag="stot")
                    nc.vector.tensor_add(stot[:], ssum[0:1, 0:1], ssum[0:1, 1:2])
                    rz = small.tile([1, 1], f32, tag="rz")
                    nc.vector.reciprocal(rz[:], stot[:])
                    prev = {"u_rowr": u_rowr, "u_row": u_row, "rz": rz, "xn": xn_cur}

    nc.compile()
    return nc


def _get_nc(bl=BL):
    if bl not in _CACHE:
        _CACHE[bl] = build(bl)
    return _CACHE[bl]


def _prep_inputs(x, hs, We, be, Wd, bd, ws):
    """Host-side relayout/quantization for one core's shard."""
    import ml_dtypes

    f8 = ml_dtypes.float8_e4m3
    bf16 = ml_dtypes.bfloat16
    bl = x.shape[0]

    # x^T chunk view: [bl, T, NH, P] -> [bl, P(h), chunk, T]
    X8 = np.ascontiguousarray(
        x.astype(f8).reshape(bl, T, NH, P).transpose(0, 3, 2, 1)
    )
    X16 = np.ascontiguousarray(
        x.astype(bf16).reshape(bl, T, NH, P).transpose(0, 3, 2, 1)
    )
    xp = np.empty((bl, P, NPAIR, 2, T), dtype=f8)
    for c in range(NPAIR):
        xp[:, :, c, 0, :] = X8[:, :, 2 * c, :]
        xp[:, :, c, 1, :] = X8[:, :, 2 * c + 1, :]
    xb = np.ascontiguousarray(X16[:, :, K_PAIR:, :])

    xn = np.ascontiguousarray(
        x.astype(bf16).reshape(bl, NT, P, H).transpose(0, 2, 1, 3)
    )

    Wa = (We * 256.0).astype(f8)
    # [o, h] -> [ch, P(h), ob, P(o)]: lhsT block layout
    WaT = Wa.reshape(NO, P, NH, P).transpose(2, 3, 0, 1)
    wp = np.empty((P, NO, NPAIR, 2, P), dtype=f8)
    for ob in range(NO):
        for c in range(NPAIR):
            wp[:, ob, c, 0, :] = WaT[2 * c, :, ob, :]
            wp[:, ob, c, 1, :] = WaT[2 * c + 1, :, ob, :]
    # bf16 chunks, scaled by 256 to share the fp8 PSUM scale
    W16T = (We * 256.0).astype(bf16).reshape(NO, P, NH, P).transpose(3, 2, 0, 1)
    wb = np.ascontiguousarray(W16T[:, K_PAIR:]).reshape(P, NBF, H)

    wdT = np.ascontiguousarray(
        Wd.T.astype(bf16).reshape(NH, P, H).transpose(1, 0, 2)
    )
    hsT = np.ascontiguousarray(
        hs.astype(bf16).reshape(bl, NH, P).transpose(2, 1, 0).reshape(P, NH * bl)
    )
    ws_col = np.ascontiguousarray(ws.astype(bf16).reshape(NH, P).T)
    be_col = np.ascontiguousarray(be.reshape(NH, P).T.astype(np.float32))
    bd_col = np.ascontiguousarray(bd.reshape(NH, P).T.astype(np.float32))
    return {
        "x_pairs": xp, "x_bf": xb, "x_nat": xn, "w_pairs": wp, "w_bf": wb,
        "wdT": wdT, "hsT": hsT, "ws_col": ws_col, "be_col": be_col,
        "bd_col": bd_col,
    }


def kernel(**inputs):
    from concourse.bass_utils import run_bass_kernel_spmd

    x = np.ascontiguousarray(np.asarray(inputs["spatial_feats"], dtype=np.float32))
    hs = np.ascontiguousarray(np.asarray(inputs["hidden_state"], dtype=np.float32))
    We = np.asarray(inputs["W_enc"], dtype=np.float32)
    be = np.asarray(inputs["b_enc"], dtype=np.float32)
    Wd = np.asarray(inputs["W_dec"], dtype=np.float32)
    bd = np.asarray(inputs["b_dec"], dtype=np.float32)
    ws = np.asarray(inputs["w_score"], dtype=np.float32)

    nc = _get_nc()
    in_maps = []
    shared = None
    for i in range(NCORES):
        m = _prep_inputs(
            x[i * BL : (i + 1) * BL], hs[i * BL : (i + 1) * BL], We, be, Wd, bd, ws
        )
        if shared is None:
            shared = {k: m[k] for k in
                      ("w_pairs", "w_bf", "wdT", "ws_col", "be_col", "bd_col")}
        else:
            m.update(shared)  # identical across cores; reuse arrays
        in_maps.append(m)
    res = run_bass_kernel_spmd(nc, in_maps, core_ids=list(range(NCORES)))
    global LAST_RESULT
    LAST_RESULT = res
    ctx = np.concatenate([res.results[i]["out_ctx"] for i in range(NCORES)], axis=0)
    w = np.concatenate([res.results[i]["out_w"] for i in range(NCORES)], axis=0)
    return (ctx, w)


# revision 65
# speedup vs baseline: 1.0197x; 1.0197x over previous
"""Bahdanau-attention kernel for Trainium2, data-parallel over 8 NeuronCores.

Per core (B_local=8, T=1024, H=1024), per batch b:
  encT[o,t] = sum_h W_enc[o,h] * x[t,h]   (PE: K_PAIR h-chunks as fp8 e4m3
      DoubleRow pairs -- 2 K-tiles of 128 per call at 2 cols/cycle -- the
      rest as exact bf16 calls. All W scaled by 256 -> tanh scale 2^-8.)
  energyT   = tanh(encT*2^-8 + (W_dec h + b_dec + b_enc)[o])  (ScalarE)
  scores[t] = sum_o w_score[o] * energyT[o,t]   (PE, bf16)
  weights   = softmax(scores)   (exp on ScalarE straight from PSUM,
      weight columnization via K=1 matmuls against a ones row)
  context   = sum_t weights[t] * x[t,:]         (PE, bf16 natural-layout x)
b_score dropped: softmax shift-invariance cancels it in both outputs.
All transposes are done host-side (xT fp8 pairs, W_encT fp8, W_decT/hsT
bf16, x natural bf16) -- no PE transposes in steady state.
"""

import sys

if "/opt/trn_rl_repo" not in sys.path:
    sys.path.insert(0, "/opt/trn_rl_repo")

import numpy as np

B, T, H = 64, 1024, 1024
NCORES = 8
BL = B // NCORES
P = 128
NT = T // P  # t chunks
NH = H // P  # h chunks
NO = H // P  # o blocks
F = 512      # PSUM bank free size (f32)
NS = T // F
K_PAIR = 6             # h-chunks contracted as fp8 DoubleRow pairs
NPAIR = K_PAIR // 2    # fp8 pair calls (2 chunks each)
NBF = NH - K_PAIR      # remaining h-chunks in exact bf16

_CACHE = {}
LAST_RESULT = None


def build(bl=BL):
    import concourse.tile as tile
    from concourse import bacc, mybir
    from concourse.masks import make_identity

    f32 = mybir.dt.float32
    f8 = mybir.dt.float8e4
    bf = mybir.dt.bfloat16
    AF = mybir.ActivationFunctionType
    DR = mybir.MatmulPerfMode.DoubleRow

    nc = bacc.Bacc("TRN2", target_bir_lowering=False, debug=False, num_devices=NCORES)
    xp_d = nc.declare_dram_parameter("x_pairs", [bl, P, NPAIR, 2, T], f8, isOutput=False)
    xb_d = nc.declare_dram_parameter("x_bf", [bl, P, NBF, T], bf, isOutput=False)
    xn_d = nc.declare_dram_parameter("x_nat", [bl, P, NT, H], bf, isOutput=False)
    wp_d = nc.declare_dram_parameter("w_pairs", [P, NO, NPAIR, 2, P], f8, isOutput=False)
    wb_d = nc.declare_dram_parameter("w_bf", [P, NBF, H], bf, isOutput=False)
    wdT_d = nc.declare_dram_parameter("wdT", [P, NH, H], bf, isOutput=False)
    hsT_d = nc.declare_dram_parameter("hsT", [P, NH * bl], bf, isOutput=False)
    ws_d = nc.declare_dram_parameter("ws_col", [P, NH], bf, isOutput=False)
    be_d = nc.declare_dram_parameter("be_col", [P, NH], f32, isOutput=False)
    bd_d = nc.declare_dram_parameter("bd_col", [P, NH], f32, isOutput=False)
    ctx_d = nc.declare_dram_parameter("out_ctx", [bl, H], f32, isOutput=True)
    wout_d = nc.declare_dram_parameter("out_w", [bl, T], f32, isOutput=True)

    with tile.TileContext(nc) as tc:
        with (
            tc.tile_pool(name="const", bufs=1) as const,
            tc.tile_pool(name="xp", bufs=2) as xp_pool,
            tc.tile_pool(name="xn", bufs=3) as xn_pool,
            tc.tile_pool(name="eT", bufs=6) as eT_pool,
            tc.tile_pool(name="rows", bufs=2) as rows,
            tc.tile_pool(name="small", bufs=2) as small,
            tc.tile_pool(name="mmps", bufs=4, space="PSUM") as mm_ps,
            tc.tile_pool(name="scps", bufs=2, space="PSUM") as sc_ps,
            tc.tile_pool(name="ctxps", bufs=2, space="PSUM") as ctx_ps,
        ):
            # ---- constants built first (gpsimd/vector heads, before DMA issues) ----
            ident_f = const.tile([P, P], f32, tag="identf")
            make_identity(nc, ident_f[:])
            ones_f = const.tile([1, NT], f32, tag="onesf")
            nc.vector.memset(ones_f[:], 1.0)
            ones8 = const.tile([1, NT], bf, tag="ones8")
            nc.vector.tensor_copy(ones8[:], ones_f[:])

            # ---- weights/batch-0 DMA ----
            # DMA bw (~220GB/s) is striped across hw engines and shared
            # proportionally among active queues: schedule two symmetric
            # queues in global priority order (enc-pair operands, wdT for the
            # dec bias, bf16-chunk operands, then batch-0 x_nat).
            hsT = const.tile([P, NH * bl], bf, tag="hsT")
            nc.sync.dma_start(hsT[:], hsT_d[:, :])
            wTt = const.tile([P, NO, NPAIR, 2, P], f8, tag="wenc")
            nc.sync.dma_start(wTt[:], wp_d[:, :, :, :, :])
            xp0 = xp_pool.tile([P, NPAIR, 2, T], f8, tag="xp")
            nc.sync.dma_start(xp0[:], xp_d[0, :, :, :, :])
            wBt = const.tile([P, NBF, H], bf, tag="wbf")
            nc.sync.dma_start(wBt[:], wb_d[:, :, :])
            xb0 = xp_pool.tile([P, NBF, T], bf, tag="xb")
            nc.sync.dma_start(xb0[:], xb_d[0, :, :, :])
            wdTt = const.tile([P, NH, H], bf, tag="wdT")
            nc.scalar.dma_start(wdTt[:, 0:5, :], wdT_d[:, 0:5, :])
            ws_sb = const.tile([P, NH], bf, tag="ws")
            nc.gpsimd.dma_start(ws_sb[:], ws_d[:, :])
            be_sb = const.tile([P, NH], f32, tag="be")
            nc.gpsimd.dma_start(be_sb[:], be_d[:, :])
            bd_sb = const.tile([P, NH], f32, tag="bd")
            nc.gpsimd.dma_start(bd_sb[:], bd_d[:, :])
            nc.gpsimd.dma_start(wdTt[:, 5:8, :], wdT_d[:, 5:8, :])
            bsum = const.tile([P, NH], f32, tag="bsum")
            nc.vector.tensor_add(bsum[:], be_sb[:], bd_sb[:])

            # ---- PE warmup: keep the array busy so HAM ramps during DMA wait
            warm_ps = mm_ps.tile([P, P], f32, tag="mmps", name="warm")
            for _ in range(16):
                nc.tensor.transpose(warm_ps[:], ident_f[:], ident_f[:])

            # ---- per-batch x DMA ----
            def dma_xpb(it):
                xp_t = xp_pool.tile([P, NPAIR, 2, T], f8, tag="xp")
                nc.sync.dma_start(xp_t[:], xp_d[it, :, :, :, :])
                xb_t = xp_pool.tile([P, NBF, T], bf, tag="xb")
                nc.sync.dma_start(xb_t[:], xb_d[it, :, :, :])
                return (xp_t, xb_t)

            def dma_xn(it):
                xn_t = xn_pool.tile([P, NT, H], bf, tag="xn")
                nc.gpsimd.dma_start(xn_t[:], xn_d[it, :, :, :])
                return xn_t

            xp_cur = (xp0, xb0)
            xn0 = xn_pool.tile([P, NT, H], bf, tag="xn")
            nc.gpsimd.dma_start(xn0[:, 0:4, :], xn_d[0, :, 0:4, :])
            nc.gpsimd.dma_start(xn0[:, 4:8, :], xn_d[0, :, 4:8, :])
            xn_cur = xn0

            bias_all = const.tile([P, NO * bl], f32, tag="bias_all")

            # ---- helpers ----
            def enc_pairs(ob, xp_t):
                psE = [mm_ps.tile([P, F], f32, tag="mmps", name=f"psE{ob}_{h}")
                       for h in range(NS)]
                for c in range(NPAIR):
                    for half in range(NS):
                        nc.tensor.matmul(
                            psE[half][:],
                            wTt[:, ob, c, :, :],
                            xp_t[:, c, :, half * F : (half + 1) * F],
                            start=(c == 0),
                            stop=False,
                            perf_mode=DR,
                        )
                return psE

            def enc_bf16(ob, xb_t, psE):
                for j in range(NBF):
                    for half in range(NS):
                        nc.tensor.matmul(
                            psE[half][:],
                            wBt[:, j, ob * P : (ob + 1) * P],
                            xb_t[:, j, half * F : (half + 1) * F],
                            start=False,
                            stop=(j == NBF - 1),
                        )

            def enc_mm(ob, x_cur):
                psE = enc_pairs(ob, x_cur[0])
                enc_bf16(ob, x_cur[1], psE)
                return psE

            def enc_act(it, ob, psE):
                es = []
                for half in range(NS):
                    e = eT_pool.tile([P, F], bf, tag="eT")
                    nc.scalar.activation(
                        e[:], psE[half][:], AF.Tanh,
                        bias=bias_all[:, ob * bl + it : ob * bl + it + 1],
                        scale=1.0 / 256.0,
                    )
                    es.append(e)
                return es

            def enc_ob(it, ob, x_cur):
                return enc_act(it, ob, enc_mm(ob, x_cur))

            def dec_bias():
                """dec[b,o] = sum_h hs[b,h] Wd[o,h]; transpose into bias_all."""
                dec_sb = const.tile([bl, H], f32, tag="decsb")
                psd = [sc_ps.tile([P, F], f32, tag="scps", name=f"psd{h}")
                       for h in range(NS)]
                for c in range(NH):
                    for half in range(NS):
                        nc.tensor.matmul(
                            psd[half][0:bl, :],
                            hsT[:, c * bl : (c + 1) * bl],
                            wdTt[:, c, half * F : (half + 1) * F],
                            start=(c == 0),
                            stop=(c == NH - 1),
                        )
                for half in range(NS):
                    nc.vector.tensor_copy(
                        dec_sb[:, half * F : (half + 1) * F], psd[half][0:bl, :]
                    )
                # preamble-only; shares the context pool's banks (context
                # first runs an iteration later, after the bias adds)
                psT = ctx_ps.tile([P, NO * bl], f32, tag="ctxps")
                for ob in range(NO):
                    nc.tensor.transpose(
                        psT[:, ob * bl : (ob + 1) * bl],
                        dec_sb[:, ob * P : (ob + 1) * P], ident_f[0:bl, 0:bl]
                    )
                for ob in range(NO):
                    nc.vector.tensor_scalar_add(
                        bias_all[:, ob * bl : (ob + 1) * bl],
                        psT[:, ob * bl : (ob + 1) * bl],
                        bsum[:, ob : ob + 1],
                    )

            def score_ob(ob, ets, pss):
                for half in range(NS):
                    nc.tensor.matmul(
                        pss[half][:],
                        ws_sb[:, ob : ob + 1],
                        ets[ob][half][:],
                        start=(ob == 0),
                        stop=(ob == NO - 1),
                    )

            # ---- main pipeline ----
            prev = None
            for it in range(bl + 1):
                if 0 < it < bl:
                    xp_cur = dma_xpb(it)
                    xn_cur = dma_xn(it)

                if it == 0:
                    # batch-0 enc matmuls for ob0/ob1 run while wdT still
                    # streams in; dec lands in between, then tanh unblocks.
                    psE0 = enc_mm(0, xp_cur)
                    psE1 = enc_mm(1, xp_cur)
                    dec_bias()
                    ets = [enc_act(0, 0, psE0), enc_act(0, 1, psE1)]
                elif it < bl:
                    # o-block 0 first: gives PE work while prev softmax drains
                    ets = [enc_ob(it, 0, xp_cur)]

                if it >= 1:
                    st = prev
                    # columnize exp-weights: uT[:, c] = u[c*128:(c+1)*128]
                    psw = mm_ps.tile([P, NT * NT], f32, tag="mmps")
                    for c in range(NT):
                        nc.tensor.matmul(
                            psw[:, c * NT : (c + 1) * NT],
                            st["u_rowr"][0:1, c * P : (c + 1) * P],
                            ones8[:],
                            start=(c == 0),
                            stop=(c == NT - 1),
                        )
                    uT = small.tile([P, NT], bf, tag="uT")
                    nc.vector.tensor_copy(uT[:], psw[:, 0 : NT * NT : NT])
                    ctx_row = rows.tile([1, H], f32, tag="ctxrow")
                    for half in range(NS):
                        pc = ctx_ps.tile([1, F], f32, tag="ctxps")
                        for c in range(NT):
                            nc.tensor.matmul(
                                pc[:],
                                uT[:, c : c + 1],
                                st["xn"][:, c, half * F : (half + 1) * F],
                                start=(c == 0),
                                stop=(c == NT - 1),
                            )
                        nc.vector.tensor_scalar_mul(
                            ctx_row[0:1, half * F : (half + 1) * F],
                            pc[0:1, :],
                            st["rz"][0:1, 0:1],
                        )
                    nc.sync.dma_start(ctx_d[it - 1 : it, :], ctx_row[:])
                    w_row = rows.tile([1, T], f32, tag="wrow")
                    nc.vector.tensor_scalar_mul(
                        w_row[:], st["u_row"][:], st["rz"][0:1, 0:1]
                    )
                    nc.sync.dma_start(wout_d[it - 1 : it, :], w_row[:])

                if it < bl:
                    pss = [sc_ps.tile([1, F], f32, tag="scps", name=f"pss{h}")
                           for h in range(NS)]
                    # scores trail enc by two o-blocks: their tanh sems are
                    # long satisfied when the PE reaches them
                    scored = 0
                    for ob in range(len(ets), NO):
                        ets.append(enc_ob(it, ob, xp_cur))
                        while scored <= ob - 2:
                            score_ob(scored, ets, pss)
                            scored += 1
                    while scored < NO:
                        score_ob(scored, ets, pss)
                        scored += 1
                    # softmax pieces: exp straight from PSUM
                    u_rowr = rows.tile([1, T], bf, tag="urowr")
                    u_row = rows.tile([1, T], f32, tag="urow")
                    ssum = small.tile([1, NS], f32, tag="ssum")
                    for half in range(NS):
                        nc.scalar.activation(
                            u_rowr[0:1, half * F : (half + 1) * F],
                            pss[half][0:1, :], AF.Exp, bias=0.0, scale=1.0,
                        )
                    for half in range(NS):
                        nc.scalar.activation(
                            u_row[0:1, half * F : (half + 1) * F],
                            pss[half][0:1, :], AF.Exp, bias=0.0, scale=1.0,
                            accum_out=ssum[0:1, half : half + 1],
                        )
                    stot = small.tile([1, 1], f32, tag="stot")
                    nc.vector.tensor_add(stot[:], ssum[0:1, 0:1], ssum[0:1, 1:2])
                    rz = small.tile([1, 1], f32, tag="rz")
                    nc.vector.reciprocal(rz[:], stot[:])
                    prev = {"u_rowr": u_rowr, "u_row": u_row, "rz": rz, "xn": xn_cur}

    nc.compile()
    return nc


def _get_nc(bl=BL):
    if bl not in _CACHE:
        _CACHE[bl] = build(bl)
    return _CACHE[bl]


def _prep_inputs(x, hs, We, be, Wd, bd, ws):
    """Host-side relayout/quantization for one core's shard."""
    import ml_dtypes

    f8 = ml_dtypes.float8_e4m3
    bf16 = ml_dtypes.bfloat16
    bl = x.shape[0]

    # x^T chunk view: [bl, T, NH, P] -> [bl, P(h), chunk, T]
    X8 = np.ascontiguousarray(
        x.astype(f8).reshape(bl, T, NH, P).transpose(0, 3, 2, 1)
    )
    X16 = np.ascontiguousarray(
        x.astype(bf16).reshape(bl, T, NH, P).transpose(0, 3, 2, 1)
    )
    xp = np.empty((bl, P, NPAIR, 2, T), dtype=f8)
    for c in range(NPAIR):
        xp[:, :, c, 0, :] = X8[:, :, 2 * c, :]
        xp[:, :, c, 1, :] = X8[:, :, 2 * c + 1, :]
    xb = np.ascontiguousarray(X16[:, :, K_PAIR:, :])

    xn = np.ascontiguousarray(
        x.astype(bf16).reshape(bl, NT, P, H).transpose(0, 2, 1, 3)
    )

    Wa = (We * 256.0).astype(f8)
    # [o, h] -> [ch, P(h), ob, P(o)]: lhsT block layout
    WaT = Wa.reshape(NO, P, NH, P).transpose(2, 3, 0, 1)
    wp = np.empty((P, NO, NPAIR, 2, P), dtype=f8)
    for ob in range(NO):
        for c in range(NPAIR):
            wp[:, ob, c, 0, :] = WaT[2 * c, :, ob, :]
            wp[:, ob, c, 1, :] = WaT[2 * c + 1, :, ob, :]
    # bf16 chunks, scaled by 256 to share the fp8 PSUM scale
    W16T = (We * 256.0).astype(bf16).reshape(NO, P, NH, P).transpose(3, 2, 0, 1)
    wb = np.ascontiguousarray(W16T[:, K_PAIR:]).reshape(P, NBF, H)

    wdT = np.ascontiguousarray(
        Wd.T.astype(bf16).reshape(NH, P, H).transpose(1, 0, 2)
    )
    hsT = np.ascontiguousarray(
        hs.astype(bf16).reshape(bl, NH, P).transpose(2, 1, 0).reshape(P, NH * bl)
    )
    ws_col = np.ascontiguousarray(ws.astype(bf16).reshape(NH, P).T)
    be_col = np.ascontiguousarray(be.reshape(NH, P).T.astype(np.float32))
    bd_col = np.ascontiguousarray(bd.reshape(NH, P).T.astype(np.float32))
    return {
        "x_pairs": xp, "x_bf": xb, "x_nat": xn, "w_pairs": wp, "w_bf": wb,
        "wdT": wdT, "hsT": hsT, "ws_col": ws_col, "be_col": be_col,
        "bd_col": bd_col,
    }


def kernel(**inputs):
    from concourse.bass_utils import run_bass_kernel_spmd

    x = np.ascontiguousarray(np.asarray(inputs["spatial_feats"], dtype=np.float32))
    hs = np.ascontiguousarray(np.asarray(inputs["hidden_state"], dtype=np.float32))
    We = np.asarray(inputs["W_enc"], dtype=np.float32)
    be = np.asarray(inputs["b_enc"], dtype=np.float32)
    Wd = np.asarray(inputs["W_dec"], dtype=np.float32)
    bd = np.asarray(inputs["b_dec"], dtype=np.float32)
    ws = np.asarray(inputs["w_score"], dtype=np.float32)

    nc = _get_nc()
    in_maps = []
    shared = None
    for i in range(NCORES):
        m = _prep_inputs(
            x[i * BL : (i + 1) * BL], hs[i * BL : (i + 1) * BL], We, be, Wd, bd, ws
        )
        if shared is None:
            shared = {k: m[k] for k in
                      ("w_pairs", "w_bf", "wdT", "ws_col", "be_col", "bd_col")}
        else:
            m.update(shared)  # identical across cores; reuse arrays
        in_maps.append(m)
    res = run_bass_kernel_spmd(nc, in_maps, core_ids=list(range(NCORES)))
    global LAST_RESULT
    LAST_RESULT = res
    ctx = np.concatenate([res.results[i]["out_ctx"] for i in range(NCORES)], axis=0)
    w = np.concatenate([res.results[i]["out_w"] for i in range(NCORES)], axis=0)
    return (ctx, w)
